# revision 5
# baseline (speedup 1.0000x reference)
"""DSSM (S4D-style) FFT-convolution kernel for Trainium2, 8 NeuronCores.

Math: y[b,h,:] = causal_conv(u_masked[b,h,:], K[h,:]) + D[h]*u_masked, masked,
where K[h,l] = 2*Re(sum_n Cs[h,n] * w[h,n]^l), w = exp(dt*A).

Algorithm (chunked state-space, T=256, J=16 chunks, N=64 complex states),
restructured for DMA/compute overlap:
  phase A:  A_j = V^T u_j (Vandermonde projection), split by chunk-half jh
            so the scan can start before all of u has arrived
  phase B:  S_j = w^T.S_{j-1} + A_j (complex scan, 15 steps, DVE bf16)
  phase C:  transposed intra-chunk Toeplitz + state output projection:
            out[tau', subblock] = Q0^T u_sb + Q1^T u_prev_sb + Wout^T S
            (columns = chunk sub-blocks, so dead chunks prune matmul cols)
Sharding: H=256 channels split across 8 cores (32 each). Host does masking,
batch length-sorting, layout transforms, bf16 casts, and final unshard+mask.
Ragged lengths: batch sorted by length desc; per-chunk-half batch counts
BH[jh] prune u DMA, matmul widths, scan widths, and y stores.
DMA order: pre/pim, u-jh0, V (4 pieces), u-jh1, wts (8 pieces), y out
(8 pieces) -- compute is emitted to overlap the DMA stream.
"""

import numpy as np
import ml_dtypes

import concourse.bass as bass
import concourse.bacc as bacc
import concourse.mybir as mybir
import concourse.tile as tile
from concourse.bass_utils import run_bass_kernel_spmd

H, N, B, L = 256, 64, 16, 4096
NCORES = 8
HC = H // NCORES            # 32 channels per core
T, J = 256, 16              # chunk length, number of chunks
JH = 2                      # chunk halves (8 chunks each)
N2 = 2 * N                  # 128 (real+imag state rows)
SW = HC * B                 # 512: scan row width (b, h)

F32 = mybir.dt.float32
BF16 = mybir.dt.bfloat16
NP_BF16 = ml_dtypes.bfloat16


def _build_program(k_b):
    """k_b: per-(sorted)batch chunk counts, used to skip dead work at trace
    time. Correctness does not depend on them (host masks dead regions)."""
    # alive batches per chunk c (batches sorted by length desc)
    nb_c = [sum(1 for k in k_b if k > c) for c in range(J)]
    # scan step j produces S_j consumed by chunk j+1
    nb_scan = [nb_c[j + 1] for j in range(J - 1)]
    BH = [nb_c[0], nb_c[8]]            # alive batches per chunk-half
    UC = (BH[0] + BH[1]) * 512         # u cols: (vb, h, sb, jl)
    YH = (BH[0] + BH[1]) * 16          # y cols per h: (jh, sb, b, jl)
    Y0, Y1 = BH[0] * 16, BH[1] * 16    # per-h per-jh y widths

    nc = bacc.Bacc("TRN2", target_bir_lowering=False, debug=False,
                   enable_asserts=False, num_devices=NCORES)

    u_d = nc.dram_tensor("u_arr", [128, UC], BF16, kind="ExternalInput")
    v_d = nc.dram_tensor("vwts", [128, HC * 256], BF16, kind="ExternalInput")
    w_d = nc.dram_tensor("wts", [128, HC * 512], BF16, kind="ExternalInput")
    pre_d = nc.dram_tensor("p_re", [128, SW], BF16, kind="ExternalInput")
    pim_d = nc.dram_tensor("p_im_s", [128, SW], BF16, kind="ExternalInput")
    y_d = nc.dram_tensor("y", [128, HC * YH], BF16, kind="ExternalOutput")

    NG = 4                  # h-groups of 8 for v DMA/phase A staging
    HG8 = HC // NG          # 8
    NW = 8                  # wts pieces (4 h each)

    with tile.TileContext(nc) as tc:
        with (
            tc.tile_pool(name="const", bufs=1) as cpool,
            tc.tile_pool(name="scantmp", bufs=3) as spool,
            tc.tile_pool(name="ysb", bufs=3) as ypool,
            tc.tile_pool(name="psum", bufs=8, space="PSUM") as psum,
        ):
            u_t = cpool.tile([128, UC], BF16, name="u_t")
            v_t = cpool.tile([128, HC * 256], BF16, name="v_t")
            w_t = cpool.tile([128, HC * 512], BF16, name="w_t")
            pre_t = cpool.tile([128, SW], BF16, name="pre_t")
            pim_t = cpool.tile([128, SW], BF16, name="pim_t")
            a_all = cpool.tile([128, J * SW], BF16, name="a_all")
            s_all = cpool.tile([128, J * SW], BF16, name="s_all")
            s_yout = cpool.tile([128, HC * 256], BF16, name="s_yout")

            # ---- DMA program order: pp, u-jh0, v, u-jh1, wts ----
            nc.sync.dma_start(pre_t[:], pre_d[:])
            nc.sync.dma_start(pim_t[:], pim_d[:])
            U0 = BH[0] * 512
            nc.sync.dma_start(u_t[:, 0:U0], u_d[:, 0:U0])
            for g in range(NG):
                s0, s1 = g * HG8 * 256, (g + 1) * HG8 * 256
                nc.sync.dma_start(v_t[:, s0:s1], v_d[:, s0:s1])
            if BH[1] > 0:
                nc.sync.dma_start(u_t[:, U0:UC], u_d[:, U0:UC])
            for g in range(NW):
                s0, s1 = g * (HC // NW) * 512, (g + 1) * (HC // NW) * 512
                nc.sync.dma_start(w_t[:, s0:s1], w_d[:, s0:s1])

            # views
            uv = u_t[:].rearrange("p (vb hh sb jl) -> p vb hh sb jl",
                                  hh=HC, sb=2, jl=8)
            uf = u_t[:].rearrange("p (vb hh s) -> p vb hh s", hh=HC, s=16)
            a_sl = a_all[:].rearrange("p (sl b h) -> p sl b h", sl=J, b=B)
            sy4 = s_yout[:].rearrange("p (h jh b jl) -> p h jh b jl",
                                      h=HC, jh=JH, b=B)

            def vslice(h, lo, hi):
                return v_t[:, h * 256 + lo:h * 256 + hi]

            def wslice(h, lo, hi):
                return w_t[:, h * 512 + lo:h * 512 + hi]

            # zero only what correctness needs: S_{-1} slot and the
            # s_yout jl=0 stripe of jh0 (chunk 0 reads S_{-1} there);
            # all other dead regions flow only into host-masked outputs
            nc.gpsimd.memset(s_all[:, 0:SW], 0.0)
            nc.gpsimd.memset(sy4[:, :, 0, 0:BH[0], 0], 0.0)

            # ---- Phase A (jh half): A_j = V^T u_j ----
            def phase_a(jh, eng):
                nb = BH[jh]
                if nb == 0:
                    return
                boff = 0 if jh == 0 else BH[0]
                for h in range(HC):
                    aps = psum.tile([128, nb * 8], F32, name="aps", tag="ps")
                    for sb in range(2):
                        nc.tensor.matmul(
                            aps[:], vslice(h, sb * 128, sb * 128 + 128),
                            uv[:, boff:boff + nb, h, sb, :],
                            start=(sb == 0), stop=(sb == 1))
                    src = aps[:].rearrange("p (b jl) -> p jl b", jl=8)
                    dst = a_sl[:, jh * 8:(jh + 1) * 8, 0:nb, h]
                    if eng == 0:
                        nc.vector.tensor_copy(dst, src)
                    else:
                        nc.scalar.copy(dst, src)

            # ---- Phase B scan steps [j0, j1): S_j = p*S_{j-1} + A_j ----
            def scan_steps(j0, j1):
                for j in range(j0, j1):
                    if j >= J - 1 or nb_scan[j] == 0:
                        continue
                    w_ = 32 * nb_scan[j]
                    s_in = s_all[:, j * SW:j * SW + w_]
                    s_out = s_all[:, (j + 1) * SW:(j + 1) * SW + w_]
                    m_a = spool.tile([128, SW], BF16, name="m_a")
                    swp = spool.tile([128, SW], BF16, name="swp")
                    m_b = spool.tile([128, SW], BF16, name="m_b")
                    tt = spool.tile([128, SW], BF16, name="tt")
                    nc.vector.tensor_mul(m_a[:, 0:w_], pre_t[:, 0:w_], s_in)
                    nc.gpsimd.tensor_copy(swp[0:64, 0:w_], s_in[64:128])
                    nc.gpsimd.tensor_copy(swp[64:128, 0:w_], s_in[0:64])
                    nc.vector.tensor_mul(m_b[:, 0:w_], pim_t[:, 0:w_],
                                         swp[:, 0:w_])
                    nc.vector.tensor_add(tt[:, 0:w_], m_a[:, 0:w_],
                                         m_b[:, 0:w_])
                    nc.vector.tensor_add(s_out, tt[:, 0:w_],
                                         a_all[:, j * SW:j * SW + w_])
                    # mirror the new slot into the (h, jh, b, jl) layout
                    jh_w, jl_w = divmod(j + 1, 8)
                    nc.scalar.copy(
                        sy4[:, :, jh_w, 0:nb_scan[j], jl_w],
                        s_out.rearrange("p (b h) -> p h b", b=nb_scan[j]))

            phase_a(0, 0)
            scan_steps(0, 8)
            phase_a(1, 1)
            scan_steps(8, J)

            # ---- Phase C (transposed): out[tau', (sb,b,jl)] per (h, jh) ----
            # yps0 (sb0 cols) = Q0^T u_sb0 + WO0^T S
            # yps1 (sb1 cols) = Q0^T u_sb1 + Q1^T u_sb0 + WO1^T S
            ceng = [0]

            def ccopy(dst, src):
                # psum sources: only DVE/ACT can read PSUM
                e = ceng[0] % 2
                ceng[0] += 1
                if e == 0:
                    nc.vector.tensor_copy(dst, src)
                else:
                    nc.scalar.copy(dst, src)

            for g in range(NG):
                ysb = ypool.tile([128, HG8 * YH], BF16, name="ysb")
                tiles = {}
                for jh in range(JH):
                    nb = BH[jh]
                    if nb == 0:
                        continue
                    boff = 0 if jh == 0 else BH[0]
                    # u-only matmuls first (no scan dependency);
                    # one psum tile per (h, jh): [sb0 cols | sb1 cols]
                    for hi in range(HG8):
                        h = g * HG8 + hi
                        yp = psum.tile([128, nb * 16], F32, name="yp",
                                       tag="ps")
                        w8 = nb * 8
                        nc.tensor.matmul(yp[:, 0:w8], wslice(h, 0, 128),
                                         uv[:, boff:boff + nb, h, 0, :],
                                         start=True, stop=False)
                        nc.tensor.matmul(yp[:, w8:2 * w8], wslice(h, 0, 128),
                                         uv[:, boff:boff + nb, h, 1, :],
                                         start=False, stop=False,
                                         skip_group_check=True)
                        nc.tensor.matmul(yp[:, w8:2 * w8],
                                         wslice(h, 128, 256),
                                         uv[:, boff:boff + nb, h, 0, :],
                                         start=False, stop=False,
                                         skip_group_check=True)
                        tiles[(hi, jh)] = yp
                    # state matmuls + psum->sbuf copies
                    for hi in range(HG8):
                        h = g * HG8 + hi
                        yp = tiles[(hi, jh)]
                        w8 = nb * 8
                        srhs = sy4[:, h, jh, 0:nb, :]
                        nc.tensor.matmul(yp[:, 0:w8], wslice(h, 256, 384),
                                         srhs, start=False, stop=False,
                                         skip_group_check=True)
                        nc.tensor.matmul(yp[:, w8:2 * w8],
                                         wslice(h, 384, 512),
                                         srhs, start=False, stop=True,
                                         skip_group_check=True)
                        base = hi * YH + (0 if jh == 0 else Y0)
                        ccopy(ysb[:, base:base + nb * 16], yp[:])
                # store: one DMA per (g, jh) block
                gbase = g * HG8 * YH
                yv = ysb[:].rearrange("p (hh q) -> p hh q", hh=HG8)
                if BH[0] > 0:
                    nc.sync.dma_start(
                        y_d[:, gbase:gbase + HG8 * YH].rearrange(
                            "p (hh q) -> p hh q", hh=HG8)[:, :, 0:Y0],
                        yv[:, :, 0:Y0])
                if BH[1] > 0:
                    nc.sync.dma_start(
                        y_d[:, gbase:gbase + HG8 * YH].rearrange(
                            "p (hh q) -> p hh q", hh=HG8)[:, :, Y0:YH],
                        yv[:, :, Y0:YH])

    nc.compile()
    return nc


_CACHE = {}


def _get_program(k_b):
    key = tuple(k_b)
    if key not in _CACHE:
        _CACHE[key] = _build_program(k_b)
    return _CACHE[key]


def _host_precompute(log_dt, C, log_A_real, A_imag, D):
    """Returns per-h weight blocks (fp64 internally)."""
    dt = np.exp(log_dt.astype(np.float64))
    A = -np.exp(log_A_real.astype(np.float64)) + 1j * A_imag.astype(np.float64)
    dtA = A * dt[:, None]
    w = np.exp(dtA)                                   # (H,N)
    Cc = C[..., 0].astype(np.float64) + 1j * C[..., 1].astype(np.float64)
    Cs = Cc * (np.exp(dtA) - 1.0) / A                 # (H,N)

    l = np.arange(T, dtype=np.float64)
    K = 2.0 * np.einsum('hn,hnl->hl', Cs, np.exp(dtA[:, :, None] * l)).real
    K[:, 0] += D.astype(np.float64)

    sig = np.arange(T)
    Vc = w[:, None, :] ** (T - sig)[None, :, None]    # (H,T,N)
    V_real = np.concatenate([Vc.real, Vc.imag], axis=2)  # (H,T,2N)

    tau = np.arange(T)
    Wc = Cs[:, :, None] * w[:, :, None] ** tau        # (H,N,T)
    W_real = np.concatenate([2 * Wc.real, -2 * Wc.imag], axis=1)  # (H,2N,T)

    p = w ** T                                        # (H,N)

    # Qrow0[h, sig', tau] = K[h, tau - sig'] for tau >= sig' else 0
    sp = np.arange(128)
    d = tau[None, :] - sp[:, None]                    # (128, 256)
    Qrow0 = np.where((d >= 0)[None], K[:, np.clip(d, 0, T - 1)], 0.0)
    return Qrow0, V_real, W_real, p


def kernel(u, length, log_dt, C, log_A_real, A_imag, D, **_unused):
    u = np.asarray(u, dtype=np.float32)
    length = np.asarray(length).astype(np.int64)
    mask = (np.arange(L)[None, :] < length[:, None])
    u_m = (u * mask[:, None, :]).astype(np.float32)

    # sort batches by length desc (stable) so dead work is a suffix
    perm = np.argsort(-length, kind="stable")
    k_b = [int(min(J, (int(length[b]) + T - 1) // T)) for b in perm]
    nb_c = [sum(1 for k in k_b if k > c) for c in range(J)]
    BH = [nb_c[0], nb_c[8]]
    YH = (BH[0] + BH[1]) * 16
    Y0 = BH[0] * 16

    Qrow0, V_real, W_real, p = _host_precompute(
        np.asarray(log_dt), np.asarray(C), np.asarray(log_A_real),
        np.asarray(A_imag), np.asarray(D))

    # phase-A weights per h: [128, 256] = [V0 | V1]; phase-C: [Qrow0 | Wout]
    vwts = np.empty((H, 128, 256), dtype=np.float64)
    vwts[:, :, 0:128] = V_real[:, 0:128, :]      # lhsT [sig', n2]
    vwts[:, :, 128:256] = V_real[:, 128:256, :]
    vwts = vwts.astype(NP_BF16)
    wts = np.empty((H, 128, 512), dtype=np.float64)
    wts[:, :, 0:256] = Qrow0                     # [Q0 | Q1]
    wts[:, :, 256:512] = W_real                  # [WO0 | WO1] (rhs [n2, tau])
    wts = wts.astype(NP_BF16)

    # p tiles: [n2, (b,h)] with re duplicated on both halves; im sign-split
    p_re = np.empty((128, H), dtype=np.float32)
    p_im = np.empty((128, H), dtype=np.float32)
    p_re[0:64] = p.real.T
    p_re[64:128] = p.real.T
    p_im[0:64] = -p.imag.T
    p_im[64:128] = p.imag.T

    # u layout: (b,h,l) -> [sig', (jh, b<BH[jh], h, sb, jl)]
    u_s = u_m[perm]
    big = u_s.reshape(B, H, JH, 8, 2, 128).transpose(5, 2, 0, 1, 4, 3)
    big = np.ascontiguousarray(big).astype(NP_BF16)  # (sig, jh, b, h, sb, jl)

    nc = _get_program(k_b)
    in_maps = []
    for c in range(NCORES):
        hs = slice(c * HC, (c + 1) * HC)
        parts = [big[:, 0, 0:BH[0], hs].reshape(128, BH[0] * 512)]
        if BH[1] > 0:
            parts.append(big[:, 1, 0:BH[1], hs].reshape(128, BH[1] * 512))
        in_maps.append({
            "u_arr": np.ascontiguousarray(np.concatenate(parts, axis=1)),
            "vwts": np.ascontiguousarray(
                vwts[hs].transpose(1, 0, 2).reshape(128, HC * 256)),
            "wts": np.ascontiguousarray(
                wts[hs].transpose(1, 0, 2).reshape(128, HC * 512)),
            "p_re": np.ascontiguousarray(
                np.tile(p_re[:, hs], (1, B))).astype(NP_BF16),
            "p_im_s": np.ascontiguousarray(
                np.tile(p_im[:, hs], (1, B))).astype(NP_BF16),
        })

    res = run_bass_kernel_spmd(nc, in_maps, core_ids=list(range(NCORES)))

    # unshard: y_d [128 tau', (g, h8, [jh0: sb,b,jl | jh1: sb,b,jl])]
    y_sorted = np.zeros((B, H, L), dtype=np.float32)
    for c in range(NCORES):
        yc = res.results[c]["y"].astype(np.float32)   # [128, HC*YH]
        yc = yc.reshape(128, HC, YH)                  # (tau, h, q)
        for jh in range(JH):
            nb = BH[jh]
            if nb == 0:
                continue
            q0 = 0 if jh == 0 else Y0
            blk = yc[:, :, q0:q0 + nb * 16].reshape(128, HC, 2, nb, 8)
            # (tau, h, sb, b, jl) -> (b, h, jl, sb, tau)
            tmp = blk.transpose(3, 1, 4, 2, 0).reshape(nb, HC, 2048)
            y_sorted[0:nb, c * HC:(c + 1) * HC,
                     jh * 2048:(jh + 1) * 2048] = tmp
    inv = np.empty(B, dtype=np.int64)
    inv[perm] = np.arange(B)
    y = y_sorted[inv]
    # np.where (not multiply): skipped regions may hold garbage bits
    # (incl. NaN) when buffers arrive non-zeroed
    y = np.where(mask[:, None, :], y, np.float32(0.0))
    return np.ascontiguousarray(y)


# revision 8
# speedup vs baseline: 1.0139x; 1.0139x over previous
"""DSSM (S4D-style) FFT-convolution kernel for Trainium2, 8 NeuronCores.

Math: y[b,h,:] = causal_conv(u_masked[b,h,:], K[h,:]) + D[h]*u_masked, masked,
where K[h,l] = 2*Re(sum_n Cs[h,n] * w[h,n]^l), w = exp(dt*A).

Algorithm (chunked state-space, T=256, J=16 chunks, N=64 complex states),
restructured for DMA/compute overlap:
  phase A:  A_j = V^T u_j (Vandermonde projection), split by chunk-half jh
            so the scan can start before all of u has arrived
  phase B:  S_j = w^T.S_{j-1} + A_j (complex scan, 15 steps, DVE bf16)
  phase C:  transposed intra-chunk Toeplitz + state output projection:
            out[tau', subblock] = Q0^T u_sb + Q1^T u_prev_sb + Wout^T S
            (columns = chunk sub-blocks, so dead chunks prune matmul cols)
Sharding: H=256 channels split across 8 cores (32 each). Host does masking,
batch length-sorting, layout transforms, bf16 casts, and final unshard+mask.
Ragged lengths: batch sorted by length desc; per-chunk-half batch counts
BH[jh] prune u DMA, matmul widths, scan widths, and y stores.
DMA order: pre/pim, u-jh0, V (4 pieces), u-jh1, wts (8 pieces), y out
(8 pieces) -- compute is emitted to overlap the DMA stream.
"""

import numpy as np
import ml_dtypes

import concourse.bass as bass
import concourse.bacc as bacc
import concourse.mybir as mybir
import concourse.tile as tile
from concourse.bass_utils import run_bass_kernel_spmd

H, N, B, L = 256, 64, 16, 4096
NCORES = 8
HC = H // NCORES            # 32 channels per core
T, J = 256, 16              # chunk length, number of chunks
JH = 2                      # chunk halves (8 chunks each)
N2 = 2 * N                  # 128 (real+imag state rows)
SW = HC * B                 # 512: scan row width (b, h)

F32 = mybir.dt.float32
BF16 = mybir.dt.bfloat16
NP_BF16 = ml_dtypes.bfloat16


def _build_program(k_b):
    """k_b: per-(sorted)batch chunk counts, used to skip dead work at trace
    time. Correctness does not depend on them (host masks dead regions)."""
    # alive batches per chunk c (batches sorted by length desc)
    nb_c = [sum(1 for k in k_b if k > c) for c in range(J)]
    # scan step j produces S_j consumed by chunk j+1
    nb_scan = [nb_c[j + 1] for j in range(J - 1)]
    BH = [nb_c[0], nb_c[8]]            # alive batches per chunk-half
    UC = (BH[0] + BH[1]) * 512         # u cols: (vb, h, sb, jl)
    YH = (BH[0] + BH[1]) * 16          # y cols per h: (jh, sb, b, jl)
    Y0, Y1 = BH[0] * 16, BH[1] * 16    # per-h per-jh y widths

    nc = bacc.Bacc("TRN2", target_bir_lowering=False, debug=False,
                   enable_asserts=False, num_devices=NCORES)

    u_d = nc.dram_tensor("u_arr", [128, UC], BF16, kind="ExternalInput")
    v_d = nc.dram_tensor("vwts", [128, HC * 256], BF16, kind="ExternalInput")
    w_d = nc.dram_tensor("wts", [128, HC * 512], BF16, kind="ExternalInput")
    pre_d = nc.dram_tensor("p_re", [128, SW], BF16, kind="ExternalInput")
    pim_d = nc.dram_tensor("p_im_s", [128, SW], BF16, kind="ExternalInput")
    y_d = nc.dram_tensor("y", [128, HC * YH], BF16, kind="ExternalOutput")

    NG = 4                  # h-groups of 8 for v DMA/phase A staging
    HG8 = HC // NG          # 8
    NW = 8                  # wts pieces (4 h each)

    with tile.TileContext(nc) as tc:
        with (
            tc.tile_pool(name="const", bufs=1) as cpool,
            tc.tile_pool(name="scantmp", bufs=3) as spool,
            tc.tile_pool(name="ysb", bufs=4) as ypool,
            tc.tile_pool(name="psum", bufs=8, space="PSUM") as psum,
        ):
            u_t = cpool.tile([128, UC], BF16, name="u_t")
            v_t = cpool.tile([128, HC * 256], BF16, name="v_t")
            w_t = cpool.tile([128, HC * 512], BF16, name="w_t")
            pre_t = cpool.tile([128, SW], BF16, name="pre_t")
            pim_t = cpool.tile([128, SW], BF16, name="pim_t")
            a_all = cpool.tile([128, J * SW], BF16, name="a_all")
            s_all = cpool.tile([128, J * SW], BF16, name="s_all")

            # ---- DMA program order: pp, u-jh0, v, u-jh1, wts ----
            nc.sync.dma_start(pre_t[:], pre_d[:])
            nc.sync.dma_start(pim_t[:], pim_d[:])
            U0 = BH[0] * 512
            nc.sync.dma_start(u_t[:, 0:U0], u_d[:, 0:U0])
            for g in range(NG):
                s0, s1 = g * HG8 * 256, (g + 1) * HG8 * 256
                nc.sync.dma_start(v_t[:, s0:s1], v_d[:, s0:s1])
            if BH[1] > 0:
                nc.sync.dma_start(u_t[:, U0:UC], u_d[:, U0:UC])
            for g in range(NW):
                s0, s1 = g * (HC // NW) * 512, (g + 1) * (HC // NW) * 512
                nc.sync.dma_start(w_t[:, s0:s1], w_d[:, s0:s1])

            # views
            uv = u_t[:].rearrange("p (vb hh sb jl) -> p vb hh sb jl",
                                  hh=HC, sb=2, jl=8)
            uf = u_t[:].rearrange("p (vb hh s) -> p vb hh s", hh=HC, s=16)
            a_sl = a_all[:].rearrange("p (sl b h) -> p sl b h", sl=J, b=B)
            s_bs = s_all[:].rearrange("p (sl b h) -> p b sl h", sl=J, b=B)

            def vslice(h, lo, hi):
                return v_t[:, h * 256 + lo:h * 256 + hi]

            def wslice(h, lo, hi):
                return w_t[:, h * 512 + lo:h * 512 + hi]

            # zero only what correctness needs: the S_{-1} slot (chunk 0's
            # state input); all other dead regions flow only into
            # host-masked outputs
            nc.gpsimd.memset(s_all[:, 0:SW], 0.0)

            # ---- Phase A (jh half): A_j = V^T u_j ----
            def phase_a(jh):
                nb = BH[jh]
                if nb == 0:
                    return
                boff = 0 if jh == 0 else BH[0]
                for h in range(HC):
                    aps = psum.tile([128, nb * 8], F32, name="aps", tag="ps")
                    for sb in range(2):
                        nc.tensor.matmul(
                            aps[:], vslice(h, sb * 128, sb * 128 + 128),
                            uv[:, boff:boff + nb, h, sb, :],
                            start=(sb == 0), stop=(sb == 1))
                    src = aps[:].rearrange("p (b jl) -> p jl b", jl=8)
                    dst = a_sl[:, jh * 8:(jh + 1) * 8, 0:nb, h]
                    if h % 2 == 0:
                        nc.vector.tensor_copy(dst, src)
                    else:
                        nc.scalar.copy(dst, src)

            # ---- Phase B scan steps [j0, j1): S_j = p*S_{j-1} + A_j ----
            def scan_steps(j0, j1):
                for j in range(j0, j1):
                    if j >= J - 1 or nb_scan[j] == 0:
                        continue
                    w_ = 32 * nb_scan[j]
                    s_in = s_all[:, j * SW:j * SW + w_]
                    s_out = s_all[:, (j + 1) * SW:(j + 1) * SW + w_]
                    m_a = spool.tile([128, SW], BF16, name="m_a")
                    swp = spool.tile([128, SW], BF16, name="swp")
                    m_b = spool.tile([128, SW], BF16, name="m_b")
                    tt = spool.tile([128, SW], BF16, name="tt")
                    nc.vector.tensor_mul(m_a[:, 0:w_], pre_t[:, 0:w_], s_in)
                    nc.gpsimd.tensor_copy(swp[0:64, 0:w_], s_in[64:128])
                    nc.gpsimd.tensor_copy(swp[64:128, 0:w_], s_in[0:64])
                    nc.vector.tensor_mul(m_b[:, 0:w_], pim_t[:, 0:w_],
                                         swp[:, 0:w_])
                    nc.vector.tensor_add(tt[:, 0:w_], m_a[:, 0:w_],
                                         m_b[:, 0:w_])
                    nc.vector.tensor_add(s_out, tt[:, 0:w_],
                                         a_all[:, j * SW:j * SW + w_])

            phase_a(0)
            scan_steps(0, 8)
            phase_a(1)
            scan_steps(8, J)

            # ---- Phase C (transposed): out[tau', (sb,b,jl)] per (h, jh) ----
            # per (g, jh) wave: 8 u-only matmul trios fill 8 psum banks,
            # then state matmuls close them, then psum->sbuf copies free
            # them. State matmuls read s_all directly ([b, slot] view).
            # yp cols: [sb0: Q0^T u_sb0 + WO0^T S | sb1: Q0^T u_sb1
            #           + Q1^T u_sb0 + WO1^T S]
            ceng = [0]

            def ccopy(dst, src):
                # psum sources: only DVE/ACT can read PSUM; ACT-heavy since
                # DVE owns the scan
                e = ceng[0] % 4
                ceng[0] += 1
                if e == 3:
                    nc.vector.tensor_copy(dst, src)
                else:
                    nc.scalar.copy(dst, src)

            ysbs = []
            for g in range(NG):
                ysb = ypool.tile([128, HG8 * YH], BF16, name="ysb")
                ysbs.append(ysb)
                for jh in range(JH):
                    nb = BH[jh]
                    if nb == 0:
                        continue
                    boff = 0 if jh == 0 else BH[0]
                    w8 = nb * 8
                    tiles = {}
                    for hi in range(HG8):
                        h = g * HG8 + hi
                        yp = psum.tile([128, nb * 16], F32, name="yp",
                                       tag="ps")
                        nc.tensor.matmul(yp[:, 0:w8], wslice(h, 0, 128),
                                         uv[:, boff:boff + nb, h, 0, :],
                                         start=True, stop=False)
                        nc.tensor.matmul(yp[:, w8:2 * w8], wslice(h, 0, 128),
                                         uv[:, boff:boff + nb, h, 1, :],
                                         start=False, stop=False,
                                         skip_group_check=True)
                        nc.tensor.matmul(yp[:, w8:2 * w8],
                                         wslice(h, 128, 256),
                                         uv[:, boff:boff + nb, h, 0, :],
                                         start=False, stop=False,
                                         skip_group_check=True)
                        tiles[hi] = yp
                    for hi in range(HG8):
                        h = g * HG8 + hi
                        yp = tiles[hi]
                        srhs = s_bs[:, 0:nb, jh * 8:(jh + 1) * 8, h]
                        nc.tensor.matmul(yp[:, 0:w8], wslice(h, 256, 384),
                                         srhs, start=False, stop=False,
                                         skip_group_check=True)
                        nc.tensor.matmul(yp[:, w8:2 * w8],
                                         wslice(h, 384, 512),
                                         srhs, start=False, stop=True,
                                         skip_group_check=True)
                        base = hi * YH + (0 if jh == 0 else Y0)
                        ccopy(ysb[:, base:base + nb * 16], yp[:])
            # stores last in SP program order so input DMA issue never
            # waits behind a store's semaphore
            for g in range(NG):
                ysb = ysbs[g]
                gbase = g * HG8 * YH
                yv = ysb[:].rearrange("p (hh q) -> p hh q", hh=HG8)
                ydv = y_d[:, gbase:gbase + HG8 * YH].rearrange(
                    "p (hh q) -> p hh q", hh=HG8)
                if BH[0] > 0:
                    nc.sync.dma_start(ydv[:, :, 0:Y0], yv[:, :, 0:Y0])
                if BH[1] > 0:
                    nc.sync.dma_start(ydv[:, :, Y0:YH], yv[:, :, Y0:YH])

    nc.compile()
    return nc


_CACHE = {}


def _get_program(k_b):
    key = tuple(k_b)
    if key not in _CACHE:
        _CACHE[key] = _build_program(k_b)
    return _CACHE[key]


def _host_precompute(log_dt, C, log_A_real, A_imag, D):
    """Returns per-h weight blocks (fp64 internally)."""
    dt = np.exp(log_dt.astype(np.float64))
    A = -np.exp(log_A_real.astype(np.float64)) + 1j * A_imag.astype(np.float64)
    dtA = A * dt[:, None]
    w = np.exp(dtA)                                   # (H,N)
    Cc = C[..., 0].astype(np.float64) + 1j * C[..., 1].astype(np.float64)
    Cs = Cc * (np.exp(dtA) - 1.0) / A                 # (H,N)

    l = np.arange(T, dtype=np.float64)
    K = 2.0 * np.einsum('hn,hnl->hl', Cs, np.exp(dtA[:, :, None] * l)).real
    K[:, 0] += D.astype(np.float64)

    sig = np.arange(T)
    Vc = w[:, None, :] ** (T - sig)[None, :, None]    # (H,T,N)
    V_real = np.concatenate([Vc.real, Vc.imag], axis=2)  # (H,T,2N)

    tau = np.arange(T)
    Wc = Cs[:, :, None] * w[:, :, None] ** tau        # (H,N,T)
    W_real = np.concatenate([2 * Wc.real, -2 * Wc.imag], axis=1)  # (H,2N,T)

    p = w ** T                                        # (H,N)

    # Qrow0[h, sig', tau] = K[h, tau - sig'] for tau >= sig' else 0
    sp = np.arange(128)
    d = tau[None, :] - sp[:, None]                    # (128, 256)
    Qrow0 = np.where((d >= 0)[None], K[:, np.clip(d, 0, T - 1)], 0.0)
    return Qrow0, V_real, W_real, p


def kernel(u, length, log_dt, C, log_A_real, A_imag, D, **_unused):
    u = np.asarray(u, dtype=np.float32)
    length = np.asarray(length).astype(np.int64)
    mask = (np.arange(L)[None, :] < length[:, None])
    u_m = (u * mask[:, None, :]).astype(np.float32)

    # sort batches by length desc (stable) so dead work is a suffix
    perm = np.argsort(-length, kind="stable")
    k_b = [int(min(J, (int(length[b]) + T - 1) // T)) for b in perm]
    nb_c = [sum(1 for k in k_b if k > c) for c in range(J)]
    BH = [nb_c[0], nb_c[8]]
    YH = (BH[0] + BH[1]) * 16
    Y0 = BH[0] * 16

    Qrow0, V_real, W_real, p = _host_precompute(
        np.asarray(log_dt), np.asarray(C), np.asarray(log_A_real),
        np.asarray(A_imag), np.asarray(D))

    # phase-A weights per h: [128, 256] = [V0 | V1]; phase-C: [Qrow0 | Wout]
    vwts = np.empty((H, 128, 256), dtype=np.float64)
    vwts[:, :, 0:128] = V_real[:, 0:128, :]      # lhsT [sig', n2]
    vwts[:, :, 128:256] = V_real[:, 128:256, :]
    vwts = vwts.astype(NP_BF16)
    wts = np.empty((H, 128, 512), dtype=np.float64)
    wts[:, :, 0:256] = Qrow0                     # [Q0 | Q1]
    wts[:, :, 256:512] = W_real                  # [WO0 | WO1] (rhs [n2, tau])
    wts = wts.astype(NP_BF16)

    # p tiles: [n2, (b,h)] with re duplicated on both halves; im sign-split
    p_re = np.empty((128, H), dtype=np.float32)
    p_im = np.empty((128, H), dtype=np.float32)
    p_re[0:64] = p.real.T
    p_re[64:128] = p.real.T
    p_im[0:64] = -p.imag.T
    p_im[64:128] = p.imag.T

    # u layout: (b,h,l) -> [sig', (jh, b<BH[jh], h, sb, jl)]
    u_s = u_m[perm]
    big = u_s.reshape(B, H, JH, 8, 2, 128).transpose(5, 2, 0, 1, 4, 3)
    big = np.ascontiguousarray(big).astype(NP_BF16)  # (sig, jh, b, h, sb, jl)

    nc = _get_program(k_b)
    in_maps = []
    for c in range(NCORES):
        hs = slice(c * HC, (c + 1) * HC)
        parts = [big[:, 0, 0:BH[0], hs].reshape(128, BH[0] * 512)]
        if BH[1] > 0:
            parts.append(big[:, 1, 0:BH[1], hs].reshape(128, BH[1] * 512))
        in_maps.append({
            "u_arr": np.ascontiguousarray(np.concatenate(parts, axis=1)),
            "vwts": np.ascontiguousarray(
                vwts[hs].transpose(1, 0, 2).reshape(128, HC * 256)),
            "wts": np.ascontiguousarray(
                wts[hs].transpose(1, 0, 2).reshape(128, HC * 512)),
            "p_re": np.ascontiguousarray(
                np.tile(p_re[:, hs], (1, B))).astype(NP_BF16),
            "p_im_s": np.ascontiguousarray(
                np.tile(p_im[:, hs], (1, B))).astype(NP_BF16),
        })

    res = run_bass_kernel_spmd(nc, in_maps, core_ids=list(range(NCORES)))

    # unshard: y_d [128 tau', (g, h8, [jh0: sb,b,jl | jh1: sb,b,jl])]
    y_sorted = np.zeros((B, H, L), dtype=np.float32)
    for c in range(NCORES):
        yc = res.results[c]["y"].astype(np.float32)   # [128, HC*YH]
        yc = yc.reshape(128, HC, YH)                  # (tau, h, q)
        for jh in range(JH):
            nb = BH[jh]
            if nb == 0:
                continue
            q0 = 0 if jh == 0 else Y0
            blk = yc[:, :, q0:q0 + nb * 16].reshape(128, HC, 2, nb, 8)
            # (tau, h, sb, b, jl) -> (b, h, jl, sb, tau)
            tmp = blk.transpose(3, 1, 4, 2, 0).reshape(nb, HC, 2048)
            y_sorted[0:nb, c * HC:(c + 1) * HC,
                     jh * 2048:(jh + 1) * 2048] = tmp
    inv = np.empty(B, dtype=np.int64)
    inv[perm] = np.arange(B)
    y = y_sorted[inv]
    # np.where (not multiply): skipped regions may hold garbage bits
    # (incl. NaN) when buffers arrive non-zeroed
    y = np.where(mask[:, None, :], y, np.float32(0.0))
    return np.ascontiguousarray(y)
